# revision 13
# baseline (speedup 1.0000x reference)
"""Baichuan attention prefill on 8 TRN2 NeuronCores (v3).

Tensor-parallel over heads: 5 heads per core. Per core:
  QKV projection -> causal attention (softmax without max-subtraction)
  -> AllGather of attention output across cores -> o_proj producing
  this core's 640 output dims. Host assembles [1, 2048, 5120].

Structure:
  - QKV uses full-5120 PSUM accumulation chains (40 matmuls deep) per
    output group: no SBUF f32 accumulators. Weights stream from HBM
    per quarter; x is resident per 512-seq quarter.
  - Attention chunk j is emitted right after quarter j's QKV sweeps,
    so AllGathers start early and phase-transition bubbles vanish.
  - Scores are exp'd in [128,1024] pairs (one ACT instruction per two
    key tiles, cross-bank PSUM read); causal masking of diagonal
    tiles is a DVE multiply with a 0/1 mask. The scalar engine runs
    ONLY Exp (drains/copies live on DVE) so its activation table
    never reloads.
  - Diagonal pairs are narrowed: S skips queries below the pair's
    first valid row, PV/L skip per-tile invalid queries.
  - S-score pipeline is 2 pairs deep and flows across head boundaries
    (pS pool bufs=3); po/pl accumulate in a single per-chunk PSUM
    tile reused by the 5 heads' chains back to back.
  - o_proj chunks emit after attention; outputs copied PSUM->SBUF on
    DVE and DMA'd from SBUF.
"""

import numpy as np
import ml_dtypes

import concourse.bacc as bacc
import concourse.mybir as mybir
from concourse.tile import TileContext
from concourse.bass_utils import run_bass_kernel_spmd

HID = 5120
NH = 40
HD = 128
S = 2048
N_CORES = 8
HPC = NH // N_CORES          # 5 heads per core
DPC = HPC * HD               # 640 dims per core
F32 = mybir.dt.float32
BF16 = mybir.dt.bfloat16
SCALE = 1.0 / float(np.sqrt(HD))

CHUNK = 512                  # seq quarter: QKV sweep + attention chunk
NCHUNK = S // CHUNK          # 4
NSLICE = HID // 128          # 40 contraction slices
RG = [list(range(N_CORES))]

_graph_cache = None


def _build_graph():
    nc = bacc.Bacc(name="baichuan_attn3")

    xt = nc.declare_dram_parameter("xt", [HID, S], BF16, isOutput=False)
    wqt = nc.declare_dram_parameter("wqt", [HID, DPC], BF16, isOutput=False)
    wkt = nc.declare_dram_parameter("wkt", [HID, DPC], BF16, isOutput=False)
    wvt = nc.declare_dram_parameter("wvt", [HID, DPC], BF16, isOutput=False)
    # wot_t[mt][p][ht*128+c] = o_proj_w[640c_core + 128 mt + c, 128 ht + p]
    wot_t = nc.declare_dram_parameter("wot_t", [HPC, 128, HID], BF16, isOutput=False)
    # masks01[p, t, q] = 1.0 if q >= 128 t + p else 0.0 (keep-mask, bf16)
    masks01 = nc.declare_dram_parameter("masks01", [128, 4, CHUNK], BF16, isOutput=False)
    ones = nc.declare_dram_parameter("ones", [128, 128], BF16, isOutput=False)
    out = nc.declare_dram_parameter("out", [DPC, S], F32, isOutput=True)

    warm_in = nc.dram_tensor("warm_in", [128, 8], BF16)
    warm_out = nc.dram_tensor("warm_out", [1024, 8], BF16, addr_space="Shared")
    ot_b = [nc.dram_tensor(f"ot_b{j}", [DPC, CHUNK], BF16) for j in range(NCHUNK)]
    og = [
        nc.dram_tensor(f"og{j}", [HID, CHUNK], BF16, addr_space="Shared")
        for j in range(NCHUNK)
    ]

    with TileContext(nc) as tc:
        with (
            tc.tile_pool(name="consts", bufs=1) as cstp,
            tc.tile_pool(name="qkv_sb", bufs=1) as sbp,
            tc.tile_pool(name="attn_pt", bufs=3) as ptp,
            tc.tile_pool(name="attn_linv", bufs=2) as lvp,
            tc.tile_pool(name="attn_ot", bufs=2) as otp,
            tc.tile_pool(name="psS", bufs=3, space="PSUM") as pS,
            tc.tile_pool(name="psO", bufs=1, space="PSUM") as pO,
        ):
            masks_sb = cstp.tile([128, 4, CHUNK], BF16, name="masks_sb")
            ones_sb = cstp.tile([128, 128], BF16, name="ones_sb")

            q_sb = sbp.tile([128, HPC, S], BF16, name="q_sb")
            k_sb = sbp.tile([128, HPC, S], BF16, name="k_sb")
            v_sb = sbp.tile([128, S // 128, DPC], BF16, name="v_sb")

            with (
                tc.tile_pool(name="qkv_x", bufs=16) as xqp,
                tc.tile_pool(name="qkv_w", bufs=8) as wp,
            ):
                xparts = {0: _load_xq(nc, 0, xqp, xt)}
                # warm up the CC channels behind the first x loads so the
                # gpsimd DMA queue isn't blocked at kernel start
                nc.gpsimd.collective_compute(
                    "AllGather",
                    mybir.AluOpType.bypass,
                    replica_groups=RG,
                    ins=[warm_in.ap().opt()],
                    outs=[warm_out.ap().opt()],
                )
                nc.gpsimd.dma_start(masks_sb[:, :, :], masks01[:, :, :])
                nc.gpsimd.dma_start(ones_sb[:, :], ones[:, :])
                for q in range(NCHUNK):
                    _qkv_quarter(nc, q, xparts[q], wp, pS, pO, wqt, wkt, wvt,
                                 q_sb, k_sb, v_sb)
                    if q + 1 < NCHUNK:
                        # prefetch next quarter's x before attention's ot
                        # stores queue up on the gpsimd DMA queue
                        xparts[q + 1] = _load_xq(nc, q + 1, xqp, xt)
                    _attn_chunk(nc, q, pS, pO, ptp, lvp, otp,
                                q_sb, k_sb, v_sb, masks_sb, ones_sb, ot_b, og)

            with (
                tc.tile_pool(name="op_w", bufs=1) as wcp,
                tc.tile_pool(name="op_og", bufs=3) as ogp,
                tc.tile_pool(name="op_y", bufs=3) as yp,
            ):
                wcols = {}
                for mt in range(HPC):
                    for piece in range(2):
                        wcol = wcp.tile(
                            [128, NH // 2, 128], BF16,
                            name=f"wo{mt}_{piece}", tag=f"wc{mt}_{piece}",
                        )
                        nc.sync.dma_start(
                            wcol[:, :, :],
                            wot_t[mt, :, piece * (HID // 2) : (piece + 1) * (HID // 2)]
                            .rearrange("p (a b) -> p a b", a=NH // 2),
                        )
                        wcols[(mt, piece)] = wcol
                for j in range(NCHUNK):
                    _oproj_chunk(nc, j, ogp, yp, pS, wcols, og, out)

    nc.compile()
    return nc


def _load_xq(nc, q, xqp, xt):
    """Load quarter q of x as 4 sub-tiles of 10 slices each, so consumers
    wait on 1/4 of the transfer rather than all 40 DMAs."""
    s0 = CHUNK * q
    parts = []
    for sp in range(8):
        xp = xqp.tile([128, NSLICE // 8, CHUNK], BF16, name=f"xq{q}_{sp}", tag="xq")
        nc.gpsimd.dma_start(
            xp[:, :, :],
            xt[640 * sp : 640 * (sp + 1), s0 : s0 + CHUNK]
            .rearrange("(a p) q -> p a q", a=NSLICE // 8),
        )
        parts.append(xp)
    return parts


def _qkv_quarter(nc, q, xparts, wp, pS, pO, wqt, wkt, wvt, q_sb, k_sb, v_sb):
    """Project x[:, 512q:512(q+1)] to q/k/v with full-5120 PSUM chains.
    All drains go to DVE so the scalar engine stays Exp-only."""
    s0 = CHUNK * q

    def xsl(i):
        return xparts[i // 5][:, i % 5, :]

    # Q and K sweeps: 5 groups (dt 0-4) each; 2 pS tiles + 1 pO tile
    for wsrc, dst, pname in ((wqt, q_sb, "q"), (wkt, k_sb, "k")):
        a0 = pS.tile([128, 2, CHUNK], F32, name=f"{pname}a0_{q}", tag="psS")
        a1 = pS.tile([128, 2, CHUNK], F32, name=f"{pname}a1_{q}", tag="psS")
        b0 = pO.tile([128, 2, CHUNK], F32, name=f"{pname}b0_{q}", tag="psO")
        slots = [a0[:, 0, :], a0[:, 1, :], a1[:, 0, :], a1[:, 1, :], b0[:, 0, :]]
        for i2 in range(NSLICE // 2):
            w = wp.tile([128, 2, DPC], BF16, name=f"w{pname}{q}_{i2}", tag="wrow")
            nc.sync.dma_start(
                w[:, :, :],
                wsrc[256 * i2 : 256 * (i2 + 1), :].rearrange("(a p) d -> p a d", a=2),
            )
            for a in range(2):
                i = 2 * i2 + a
                for dt in range(HPC):
                    nc.tensor.matmul(
                        slots[dt],
                        lhsT=w[:, a, 128 * dt : 128 * (dt + 1)],
                        rhs=xsl(i),
                        start=(i == 0),
                        stop=(i == NSLICE - 1),
                    )
        for dt in range(HPC):
            nc.vector.tensor_scalar_mul(dst[:, dt, s0 : s0 + CHUNK], slots[dt], 1.0)

    # V sweep: 8 groups (st 0-3 x nh 0-1) in natural [seq, dims] layout
    a0 = pS.tile([128, 2, CHUNK], F32, name=f"va0_{q}", tag="psS")
    a1 = pS.tile([128, 2, CHUNK], F32, name=f"va1_{q}", tag="psS")
    a2 = pS.tile([128, 2, CHUNK], F32, name=f"va2_{q}", tag="psS")
    b0 = pO.tile([128, 2, CHUNK], F32, name=f"vb0_{q}", tag="psO")
    vt = [a0, a1, a2, b0]
    for i2 in range(NSLICE // 2):
        w = wp.tile([128, 2, DPC], BF16, name=f"wv{q}_{i2}", tag="wrow")
        nc.sync.dma_start(
            w[:, :, :],
            wvt[256 * i2 : 256 * (i2 + 1), :].rearrange("(a p) d -> p a d", a=2),
        )
        for a in range(2):
            i = 2 * i2 + a
            for st in range(4):
                for nh in range(2):
                    nc.tensor.matmul(
                        vt[st][:, nh, 0:320],
                        lhsT=xsl(i)[:, 128 * st : 128 * (st + 1)],
                        rhs=w[:, a, 320 * nh : 320 * (nh + 1)],
                        start=(i == 0),
                        stop=(i == NSLICE - 1),
                    )
    for st in range(4):
        for nh in range(2):
            nc.vector.tensor_scalar_mul(
                v_sb[:, 4 * q + st, 320 * nh : 320 * (nh + 1)], vt[st][:, nh, 0:320], 1.0
            )


def _attn_chunk(nc, j, pS, pO, ptp, lvp, otp, q_sb, k_sb, v_sb, masks_sb,
                ones_sb, ot_b, og):
    """Attention for queries [512j, 512(j+1)), 5 heads.
    2-pair-deep S pipeline flowing across head boundaries; po/pl live in
    one per-chunk PSUM tile reused sequentially by the heads' chains."""
    nk = 4 * (j + 1)
    npair = nk // 2
    bpl = pO.tile([128, 2, CHUNK], F32, name=f"opl{j}", tag="psO")
    po = bpl[:, 0, :]
    pl = bpl[:, 1, :]

    queue = []

    def _tail(pend):
        ps, p, h = pend
        t0 = max(2 * p - 4 * j, 0) if 2 * p >= 4 * j else -1
        c0 = 128 * t0 if t0 >= 0 else 0  # narrowed start column (pair level)
        ptile = ptp.tile([128, 2, CHUNK], BF16, name=f"pt{j}_{h}_{p}", tag="pt")
        nc.scalar.activation(
            ptile[:, :, c0:CHUNK], ps[:, :, c0:CHUNK],
            mybir.ActivationFunctionType.Exp, bias=0.0, scale=SCALE,
        )
        if t0 >= 0:
            nc.vector.tensor_mul(
                ptile[:, :, c0:CHUNK], ptile[:, :, c0:CHUNK],
                masks_sb[:, t0 : t0 + 2, c0:CHUNK],
            )
        for t in range(2):
            kt = 2 * p + t
            ct = 128 * (t0 + t) if t0 >= 0 else 0  # per-tile narrowed start
            nc.tensor.matmul(
                po[:, ct:CHUNK],
                lhsT=v_sb[:, kt, 128 * h : 128 * (h + 1)],
                rhs=ptile[:, t, ct:CHUNK],
                start=(kt == 0), stop=(kt == nk - 1),
            )
            nc.tensor.matmul(
                pl[:, ct:CHUNK],
                lhsT=ones_sb[:, :],
                rhs=ptile[:, t, ct:CHUNK],
                start=(kt == 0), stop=(kt == nk - 1),
            )
        if p == npair - 1:
            # head h's accumulation done: normalize and store
            linv = lvp.tile([128, CHUNK], F32, name=f"linv{j}_{h}", tag="linv")
            nc.vector.reciprocal_approx_fast(linv[:, :], pl)
            ot = otp.tile([128, CHUNK], BF16, name=f"ot{j}_{h}", tag="ot")
            nc.vector.tensor_mul(ot[:, :], po, linv[:, :])
            nc.gpsimd.dma_start(ot_b[j][128 * h : 128 * (h + 1), :], ot[:, :])

    for h in range(HPC):
        qtile = q_sb[:, h, CHUNK * j : CHUNK * (j + 1)]
        for p in range(npair):
            ps = pS.tile([128, 2, CHUNK], F32, name=f"s{j}_{h}_{p}", tag="psS")
            diag = 2 * p >= 4 * j
            c0 = 128 * (2 * p - 4 * j) if diag else 0
            for t in range(2):
                kt = 2 * p + t
                nc.tensor.matmul(
                    ps[:, t, c0:CHUNK],
                    lhsT=k_sb[:, h, 128 * kt : 128 * (kt + 1)],
                    rhs=qtile[:, c0:CHUNK],
                    start=True,
                    stop=True,
                )
            queue.append((ps, p, h))
            if len(queue) > 2:
                _tail(queue.pop(0))
    while queue:
        _tail(queue.pop(0))

    nc.gpsimd.collective_compute(
        "AllGather",
        mybir.AluOpType.bypass,
        replica_groups=RG,
        ins=[ot_b[j].ap().opt()],
        outs=[og[j].ap().opt()],
    )


def _oproj_chunk(nc, j, ogp, yp, pS, wcols, og, out):
    """o_proj for seq chunk j: full 5120 contraction, 5 out-tiles of 128."""
    halves = []
    for piece in range(2):
        ogt = ogp.tile([128, NH // 2, CHUNK], BF16, name=f"ogt{j}_{piece}", tag="ogt")
        nc.sync.dma_start(
            ogt[:, :, :],
            og[j][2560 * piece : 2560 * (piece + 1), :]
            .rearrange("(a p) q -> p a q", a=NH // 2),
        )
        halves.append(ogt)
    for mt in range(HPC):
        ps = pS.tile([128, 2, CHUNK], F32, name=f"py{j}_{mt}", tag="psS")
        for ht in range(NH):
            piece, hh = divmod(ht, NH // 2)
            nc.tensor.matmul(
                ps[:, 0, :],
                lhsT=wcols[(mt, piece)][:, hh, :],
                rhs=halves[piece][:, hh, :],
                start=(ht == 0),
                stop=(ht == NH - 1),
            )
        ysb = yp.tile([128, CHUNK], F32, name=f"y{j}_{mt}", tag="y")
        nc.vector.tensor_scalar_mul(ysb[:, :], ps[:, 0, :], 1.0)
        nc.gpsimd.dma_start(
            out[128 * mt : 128 * (mt + 1), CHUNK * j : CHUNK * (j + 1)],
            ysb[:, :],
        )


def _to_bf16(a):
    return np.asarray(a, dtype=np.float32).astype(ml_dtypes.bfloat16)


def _prep_inputs(hidden_states, W_pack_w, o_proj_w):
    xt = _to_bf16(np.ascontiguousarray(hidden_states.reshape(S, HID).T))
    # keep-mask: masks01[p, t, q] = 1 if q >= 128 t + p else 0
    masks01 = np.zeros((128, 4, CHUNK), dtype=np.float32)
    qidx = np.arange(CHUNK)
    for t in range(4):
        for p in range(128):
            masks01[p, t, :] = (qidx >= 128 * t + p).astype(np.float32)
    masks01 = masks01.astype(ml_dtypes.bfloat16)
    ones = np.ones((128, 128), dtype=ml_dtypes.bfloat16)
    in_maps = []
    for c in range(N_CORES):
        r0 = DPC * c
        woc = o_proj_w[r0 : r0 + DPC, :]          # [640 out, 5120 in]
        wot_t = np.ascontiguousarray(
            woc.reshape(HPC, 128, NH, 128).transpose(0, 3, 2, 1).reshape(HPC, 128, HID)
        )
        in_maps.append(
            {
                "xt": xt,
                "wqt": _to_bf16(np.ascontiguousarray(W_pack_w[r0 : r0 + DPC, :].T)),
                "wkt": _to_bf16(np.ascontiguousarray(W_pack_w[HID + r0 : HID + r0 + DPC, :].T)),
                "wvt": _to_bf16(np.ascontiguousarray(W_pack_w[2 * HID + r0 : 2 * HID + r0 + DPC, :].T)),
                "wot_t": _to_bf16(wot_t),
                "masks01": masks01,
                "ones": ones,
            }
        )
    return in_maps


def run(hidden_states, W_pack_w, o_proj_w, trace=False):
    global _graph_cache
    if _graph_cache is None:
        _graph_cache = _build_graph()
    nc = _graph_cache
    in_maps = _prep_inputs(hidden_states, W_pack_w, o_proj_w)
    res = run_bass_kernel_spmd(nc, in_maps, list(range(N_CORES)), trace=trace)
    y = np.concatenate([res.results[c]["out"].T for c in range(N_CORES)], axis=1)
    return y.reshape(1, S, HID), res


def kernel(
    hidden_states,
    W_pack_w,
    o_proj_w,
    k_cache=None,
    v_cache=None,
    input_pos=None,
    attention_mask=None,
    **_unused,
):
    hidden_states = np.asarray(hidden_states, dtype=np.float32)
    W_pack_w = np.asarray(W_pack_w, dtype=np.float32)
    o_proj_w = np.asarray(o_proj_w, dtype=np.float32)
    y, _ = run(hidden_states, W_pack_w, o_proj_w, trace=False)
    return y.reshape(1, S, HID)
